# revision 9
# baseline (speedup 1.0000x reference)
"""Trainium2 Bass kernel: single-channel 11x11 same-padding 2D cross-correlation.

Problem: x [64, 1024, 1024] f32, weight [11, 11] f32 ->
         out[b,h,w] = sum_{i,j} x_pad[b, h+i-5, w+j-5] * weight[i,j]

Strategy
--------
Pure data parallel over batch: 8 images per NeuronCore across 8 cores.

Per core, the conv is computed on the TensorEngine as banded-Toeplitz
matmuls. For an output row-tile of MTILE=118 rows, the contraction dim
(SBUF partition axis) holds 128 input rows (118 + 10 halo). For each of
the 11 kernel columns j we issue one matmul:

    psum[m, n] += sum_p T_j[p, m] * xtile[p, n + j]

where T_j[p, m] = weight[p - m, j] for 0 <= p - m <= 10 (banded, built
host-side), and the rhs access pattern is just the x tile shifted by j
along the free (W) axis. 11 matmuls accumulate the full 11x11 stencil
into one PSUM tile. x is host-padded (zeros) so every DMA and matmul is
uniform: H padded by 5 on top (and to the tile grid at the bottom), W
padded by 5 on both sides.

dtype: float32r (reduced-precision fp32 matmul, 1 col/cycle at N>=256)
or bf16 (host-cast, halves input DMA bytes). PSUM accumulation is fp32
either way.
"""

import math

import numpy as np

KK = 11      # kernel size
PAD = 5      # same padding
MTILE = 118  # output rows per tile; contraction = MTILE + 2*PAD = 128
KDIM = 128   # contraction partitions
NCORES = 8

# "fp32r" | "bf16" | "fp32"
DTYPE = "fp32r"

_CACHE = {}


def build_tmats(weight, dtype_np):
    """[128, 11*118] stationary banded matrices; T_j columns m, band = kernel col j."""
    T = np.zeros((KDIM, KK * MTILE), dtype=np.float32)
    for j in range(KK):
        for d in range(KK):
            # T[m + d, j*MTILE + m] = weight[d, j]
            idx_m = np.arange(0, MTILE)
            idx_p = idx_m + d
            ok = idx_p < KDIM
            T[idx_p[ok], j * MTILE + idx_m[ok]] = weight[d, j]
    return np.ascontiguousarray(T.astype(dtype_np))


def _dt():
    import concourse.mybir as mybir
    import ml_dtypes

    if DTYPE == "fp32r":
        return mybir.dt.float32r, np.float32
    if DTYPE == "bf16":
        return mybir.dt.bfloat16, ml_dtypes.bfloat16
    return mybir.dt.float32, np.float32


def build_nc(b, h, w):
    """Bass program for one core: b images of [h, w]."""
    import concourse.mybir as mybir
    from concourse import bacc
    from concourse.tile import TileContext

    dt_mm, _ = _dt()
    ntiles = math.ceil(h / MTILE)
    hp = MTILE * (ntiles - 1) + KDIM   # host-padded H
    wp = w + 2 * PAD                   # host-padded W
    nhalf = w // 512
    assert w % 512 == 0

    nc = bacc.Bacc("TRN2", target_bir_lowering=False)
    x = nc.dram_tensor("x", (b, hp, wp), dt_mm, kind="ExternalInput")
    tm = nc.dram_tensor("tmats", (KDIM, KK * MTILE), dt_mm, kind="ExternalInput")
    out = nc.dram_tensor("out", (b, h, w), mybir.dt.float32, kind="ExternalOutput")

    with TileContext(nc) as tc:
        with (
            tc.tile_pool(name="wpool", bufs=1) as wpool,
            tc.tile_pool(name="xpool", bufs=3) as xpool,
            tc.tile_pool(name="opool", bufs=3) as opool,
            tc.tile_pool(name="psum", bufs=4, space="PSUM") as ppool,
            tc.tile_pool(name="scratch", bufs=1, space="PSUM") as spool,
        ):
            tsb = wpool.tile([KDIM, KK * MTILE], dt_mm)
            nc.sync.dma_start(tsb[:, :], tm[:, :])
            scr = spool.tile([1, 8], mybir.dt.float32)
            for img in range(b):
                for t in range(ntiles):
                    a = t * MTILE
                    xt = xpool.tile([KDIM, wp], dt_mm)
                    nc.sync.dma_start(xt[:, :], x[img, a:a + KDIM, :])
                    # Pre-touch: a 1x1 dummy matmul absorbs the DMA-completion
                    # wait on the PE queue, so real matmuls (whose fused
                    # weight-load struct has a single sync-wait slot) never
                    # carry more than one wait each.
                    nc.tensor.matmul(
                        scr[0:1, 0:2], xt[0:1, 0:1], xt[0:1, 0:2],
                        start=True, stop=True, skip_group_check=True,
                    )
                    ot = opool.tile([MTILE, w], mybir.dt.float32)
                    for half in range(nhalf):
                        ps = ppool.tile([MTILE, 512], mybir.dt.float32)
                        base = half * 512
                        for j in range(KK):
                            nc.tensor.matmul(
                                ps[:, :],
                                tsb[:, j * MTILE:(j + 1) * MTILE],
                                xt[:, base + j: base + j + 512],
                                start=(j == 0),
                                stop=(j == KK - 1),
                            )
                        nc.vector.tensor_copy(ot[:, base:base + 512], ps[:, :])
                    rows = min(MTILE, h - a)
                    nc.sync.dma_start(out[img, a:a + rows, :], ot[:rows, :])
    nc.compile()
    return nc


def _pad_input(x, h, w, dtype_np):
    """[B, hp, wp] zero-padded copy of x."""
    B = x.shape[0]
    ntiles = math.ceil(h / MTILE)
    hp = MTILE * (ntiles - 1) + KDIM
    wp = w + 2 * PAD
    xp = np.zeros((B, hp, wp), dtype=dtype_np)
    xp[:, PAD:PAD + h, PAD:PAD + w] = x
    return xp


def kernel(x, weight):
    from concourse.bass_utils import run_bass_kernel_spmd

    x = np.asarray(x)
    weight = np.asarray(weight)
    B, h, w = x.shape
    assert B % NCORES == 0
    bpc = B // NCORES
    _, dtype_np = _dt()

    key = (bpc, h, w, DTYPE)
    if key not in _CACHE:
        _CACHE[key] = build_nc(bpc, h, w)
    nc = _CACHE[key]

    xp = _pad_input(x, h, w, dtype_np)
    tm = build_tmats(weight.astype(np.float32), dtype_np)
    in_maps = [
        {"x": xp[c * bpc:(c + 1) * bpc], "tmats": tm} for c in range(NCORES)
    ]
    res = run_bass_kernel_spmd(nc, in_maps, core_ids=list(range(NCORES)))
    global _LAST_RESULTS
    _LAST_RESULTS = res
    return np.concatenate([r["out"] for r in res.results], axis=0).astype(np.float32)


def bench(x, weight, iters=20):
    """Time device execution with device-resident inputs (no donation, no
    per-iter host transfers). Returns (out, per-iter seconds list)."""
    import time

    import jax
    from jax.experimental.shard_map import shard_map
    from jax.sharding import Mesh, PartitionSpec

    import concourse.mybir as mybir
    from concourse import bass2jax

    x = np.asarray(x)
    weight = np.asarray(weight)
    B, h, w = x.shape
    bpc = B // NCORES
    _, dtype_np = _dt()
    key = (bpc, h, w, DTYPE)
    if key not in _CACHE:
        _CACHE[key] = build_nc(bpc, h, w)
    nc = _CACHE[key]

    bass2jax.install_neuronx_cc_hook()
    partition_name = nc.partition_id_tensor.name if nc.partition_id_tensor else None
    in_names, out_names, out_avals = [], [], []
    for alloc in nc.m.functions[0].allocations:
        if not isinstance(alloc, mybir.MemoryLocationSet):
            continue
        name = alloc.memorylocations[0].name
        if alloc.kind == "ExternalInput":
            if name != partition_name:
                in_names.append(name)
        elif alloc.kind == "ExternalOutput":
            out_names.append(name)
            out_avals.append(
                jax.core.ShapedArray(
                    tuple(alloc.tensor_shape), mybir.dt.np(alloc.dtype)
                )
            )
    n_params = len(in_names)
    all_in_names = in_names + out_names
    if partition_name is not None:
        all_in_names = all_in_names + [partition_name]

    def _body(*args):
        operands = list(args)
        if partition_name is not None:
            operands.append(bass2jax.partition_id_tensor())
        return tuple(
            bass2jax._bass_exec_p.bind(
                *operands,
                out_avals=tuple(out_avals),
                in_names=tuple(all_in_names),
                out_names=tuple(out_names),
                lowering_input_output_aliases=(),
                sim_require_finite=True,
                sim_require_nnan=True,
                nc=nc,
            )
        )

    devices = jax.devices()[:NCORES]
    mesh = Mesh(np.asarray(devices), ("core",))
    n_outs = len(out_names)
    fn = jax.jit(
        shard_map(
            _body,
            mesh=mesh,
            in_specs=(PartitionSpec("core"),) * (n_params + n_outs),
            out_specs=(PartitionSpec("core"),) * n_outs,
            check_rep=False,
        ),
        keep_unused=True,
    )

    xp = _pad_input(x, h, w, dtype_np)
    tm = build_tmats(weight.astype(np.float32), dtype_np)
    per_core = {"x": xp, "tmats": np.concatenate([tm[None]] * NCORES, 0).reshape(NCORES * tm.shape[0], tm.shape[1])}
    concat_in = [per_core[name] for name in in_names]
    concat_zeros = [
        np.zeros((NCORES * a.shape[0], *a.shape[1:]), a.dtype) for a in out_avals
    ]
    from jax.sharding import NamedSharding
    shard = NamedSharding(mesh, PartitionSpec("core"))
    dev_in = [jax.device_put(a, shard) for a in concat_in]
    dev_zero = [jax.device_put(a, shard) for a in concat_zeros]

    out = fn(*dev_in, *dev_zero)  # compile + warmup
    jax.block_until_ready(out)
    times = []
    for _ in range(iters):
        t0 = time.perf_counter()
        out = fn(*dev_in, *dev_zero)
        jax.block_until_ready(out)
        times.append(time.perf_counter() - t0)
    full = np.asarray(out[0]).reshape(NCORES, bpc, h, w).reshape(B, h, w)
    return full.astype(np.float32), times


# revision 13
# speedup vs baseline: 39.2974x; 39.2974x over previous
"""Trainium2 Bass kernel: single-channel 11x11 same-padding 2D cross-correlation.

Problem: x [64, 1024, 1024] f32, weight [11, 11] f32 ->
         out[b,h,w] = sum_{i,j} x_pad[b, h+i-5, w+j-5] * weight[i,j]

Strategy
--------
Pure data parallel over batch: 8 images per NeuronCore across 8 cores.

Per core, the conv is computed on the TensorEngine as banded-Toeplitz
matmuls. For an output row-tile of MTILE=118 rows, the contraction dim
(SBUF partition axis) holds 128 input rows (118 + 10 halo). For each of
the 11 kernel columns j we issue one matmul:

    psum[m, n] += sum_p T_j[p, m] * xtile[p, n + j]

where T_j[p, m] = weight[p - m, j] for 0 <= p - m <= 10 (banded, built
host-side), and the rhs access pattern is just the x tile shifted by j
along the free (W) axis. 11 matmuls accumulate the full 11x11 stencil
into one PSUM tile. x is host-padded (zeros) so every DMA and matmul is
uniform: H padded by 5 on top (and to the tile grid at the bottom), W
padded by 5 on both sides.

dtype: float32r (reduced-precision fp32 matmul, 1 col/cycle at N>=256)
or bf16 (host-cast, halves input DMA bytes). PSUM accumulation is fp32
either way.
"""

import math

import numpy as np

KK = 11      # kernel size
PAD = 5      # same padding
MTILE = 118  # output rows per tile; contraction = MTILE + 2*PAD = 128
KDIM = 128   # contraction partitions
NCORES = 8

# "fp32r" | "bf16" | "fp32"
DTYPE = "fp32r"

_CACHE = {}


def build_tmats(weight, dtype_np):
    """[128, 11*118] stationary banded matrices; T_j columns m, band = kernel col j."""
    T = np.zeros((KDIM, KK * MTILE), dtype=np.float32)
    for j in range(KK):
        for d in range(KK):
            # T[m + d, j*MTILE + m] = weight[d, j]
            idx_m = np.arange(0, MTILE)
            idx_p = idx_m + d
            ok = idx_p < KDIM
            T[idx_p[ok], j * MTILE + idx_m[ok]] = weight[d, j]
    return np.ascontiguousarray(T.astype(dtype_np))


def _dt():
    import concourse.mybir as mybir
    import ml_dtypes

    if DTYPE == "fp32r":
        return mybir.dt.float32r, np.float32
    if DTYPE == "bf16":
        return mybir.dt.bfloat16, ml_dtypes.bfloat16
    return mybir.dt.float32, np.float32


def build_nc(b, h, w, repeat=1):
    """Bass program for one core: b images of [h, w].

    repeat > 1 wraps the whole body in a hardware For-loop that redoes the
    identical work; used only for wall-clock-delta HW timing (the axon RPC
    dispatch floor is ~100 ms, far above the kernel's real runtime).
    """
    import contextlib

    import concourse.mybir as mybir
    from concourse import bacc
    from concourse.tile import TileContext

    dt_mm, _ = _dt()
    ntiles = math.ceil(h / MTILE)
    hp = MTILE * (ntiles - 1) + KDIM   # host-padded H
    wp = w + 2 * PAD                   # host-padded W
    nhalf = w // 512
    assert w % 512 == 0

    nc = bacc.Bacc("TRN2", target_bir_lowering=False)
    x = nc.dram_tensor("x", (b, hp, wp), dt_mm, kind="ExternalInput")
    tm = nc.dram_tensor("tmats", (KDIM, KK * MTILE), dt_mm, kind="ExternalInput")
    out = nc.dram_tensor("out", (b, h, w), mybir.dt.float32, kind="ExternalOutput")

    with TileContext(nc) as tc:
        with (
            tc.tile_pool(name="wpool", bufs=1) as wpool,
            tc.tile_pool(name="xpool", bufs=3) as xpool,
            tc.tile_pool(name="opool", bufs=3) as opool,
            tc.tile_pool(name="psum", bufs=4, space="PSUM") as ppool,
            tc.tile_pool(name="scratch", bufs=1, space="PSUM") as spool,
        ):
            tsb = wpool.tile([KDIM, KK * MTILE], dt_mm)
            nc.sync.dma_start(tsb[:, :], tm[:, :])
            scr = spool.tile([1, 8], mybir.dt.float32)
            loop = tc.For_i(0, repeat, 1) if repeat > 1 else contextlib.nullcontext()
            with loop:
                for img in range(b):
                    for t in range(ntiles):
                        a = t * MTILE
                        xt = xpool.tile([KDIM, wp], dt_mm)
                        nc.sync.dma_start(xt[:, :], x[img, a:a + KDIM, :])
                        # Pre-touch: a 1x1 dummy matmul absorbs the
                        # DMA-completion wait on the PE queue, so real matmuls
                        # (whose fused weight-load struct has a single
                        # sync-wait slot) never carry more than one wait each.
                        nc.tensor.matmul(
                            scr[0:1, 0:2], xt[0:1, 0:1], xt[0:1, 0:2],
                            start=True, stop=True, skip_group_check=True,
                        )
                        ot = opool.tile([MTILE, w], mybir.dt.float32)
                        for half in range(nhalf):
                            ps = ppool.tile([MTILE, 512], mybir.dt.float32)
                            base = half * 512
                            for j in range(KK):
                                nc.tensor.matmul(
                                    ps[:, :],
                                    tsb[:, j * MTILE:(j + 1) * MTILE],
                                    xt[:, base + j: base + j + 512],
                                    start=(j == 0),
                                    stop=(j == KK - 1),
                                )
                            nc.vector.tensor_copy(ot[:, base:base + 512], ps[:, :])
                        rows = min(MTILE, h - a)
                        nc.sync.dma_start(out[img, a:a + rows, :], ot[:rows, :])
    nc.compile()
    return nc


def _pad_input(x, h, w, dtype_np):
    """[B, hp, wp] zero-padded copy of x."""
    B = x.shape[0]
    ntiles = math.ceil(h / MTILE)
    hp = MTILE * (ntiles - 1) + KDIM
    wp = w + 2 * PAD
    xp = np.zeros((B, hp, wp), dtype=dtype_np)
    xp[:, PAD:PAD + h, PAD:PAD + w] = x
    return xp


def kernel(x, weight):
    from concourse.bass_utils import run_bass_kernel_spmd

    x = np.asarray(x)
    weight = np.asarray(weight)
    B, h, w = x.shape
    assert B % NCORES == 0
    bpc = B // NCORES
    _, dtype_np = _dt()

    key = (bpc, h, w, DTYPE, 1)
    if key not in _CACHE:
        _CACHE[key] = build_nc(bpc, h, w)
    nc = _CACHE[key]

    xp = _pad_input(x, h, w, dtype_np)
    tm = build_tmats(weight.astype(np.float32), dtype_np)
    in_maps = [
        {"x": xp[c * bpc:(c + 1) * bpc], "tmats": tm} for c in range(NCORES)
    ]
    res = run_bass_kernel_spmd(nc, in_maps, core_ids=list(range(NCORES)))
    global _LAST_RESULTS
    _LAST_RESULTS = res
    return np.concatenate([r["out"] for r in res.results], axis=0).astype(np.float32)


def bench(x, weight, iters=20, repeat=1):
    """Time device execution with device-resident inputs (no donation, no
    per-iter host transfers). Returns (out, per-iter seconds list)."""
    import time

    import jax
    from jax.experimental.shard_map import shard_map
    from jax.sharding import Mesh, PartitionSpec

    import concourse.mybir as mybir
    from concourse import bass2jax

    x = np.asarray(x)
    weight = np.asarray(weight)
    B, h, w = x.shape
    bpc = B // NCORES
    _, dtype_np = _dt()
    key = (bpc, h, w, DTYPE, repeat)
    if key not in _CACHE:
        _CACHE[key] = build_nc(bpc, h, w, repeat=repeat)
    nc = _CACHE[key]

    bass2jax.install_neuronx_cc_hook()
    partition_name = nc.partition_id_tensor.name if nc.partition_id_tensor else None
    in_names, out_names, out_avals = [], [], []
    for alloc in nc.m.functions[0].allocations:
        if not isinstance(alloc, mybir.MemoryLocationSet):
            continue
        name = alloc.memorylocations[0].name
        if alloc.kind == "ExternalInput":
            if name != partition_name:
                in_names.append(name)
        elif alloc.kind == "ExternalOutput":
            out_names.append(name)
            out_avals.append(
                jax.core.ShapedArray(
                    tuple(alloc.tensor_shape), mybir.dt.np(alloc.dtype)
                )
            )
    n_params = len(in_names)
    all_in_names = in_names + out_names
    if partition_name is not None:
        all_in_names = all_in_names + [partition_name]

    def _body(*args):
        operands = list(args)
        if partition_name is not None:
            operands.append(bass2jax.partition_id_tensor())
        return tuple(
            bass2jax._bass_exec_p.bind(
                *operands,
                out_avals=tuple(out_avals),
                in_names=tuple(all_in_names),
                out_names=tuple(out_names),
                lowering_input_output_aliases=(),
                sim_require_finite=True,
                sim_require_nnan=True,
                nc=nc,
            )
        )

    devices = jax.devices()[:NCORES]
    mesh = Mesh(np.asarray(devices), ("core",))
    n_outs = len(out_names)
    fn = jax.jit(
        shard_map(
            _body,
            mesh=mesh,
            in_specs=(PartitionSpec("core"),) * (n_params + n_outs),
            out_specs=(PartitionSpec("core"),) * n_outs,
            check_rep=False,
        ),
        keep_unused=True,
    )

    xp = _pad_input(x, h, w, dtype_np)
    tm = build_tmats(weight.astype(np.float32), dtype_np)
    per_core = {"x": xp, "tmats": np.concatenate([tm[None]] * NCORES, 0).reshape(NCORES * tm.shape[0], tm.shape[1])}
    concat_in = [per_core[name] for name in in_names]
    concat_zeros = [
        np.zeros((NCORES * a.shape[0], *a.shape[1:]), a.dtype) for a in out_avals
    ]
    from jax.sharding import NamedSharding
    shard = NamedSharding(mesh, PartitionSpec("core"))
    dev_in = [jax.device_put(a, shard) for a in concat_in]
    dev_zero = [jax.device_put(a, shard) for a in concat_zeros]

    out = fn(*dev_in, *dev_zero)  # compile + warmup
    jax.block_until_ready(out)
    times = []
    for _ in range(iters):
        t0 = time.perf_counter()
        out = fn(*dev_in, *dev_zero)
        jax.block_until_ready(out)
        times.append(time.perf_counter() - t0)
    full = np.asarray(out[0]).reshape(NCORES, bpc, h, w).reshape(B, h, w)
    return full.astype(np.float32), times


def bench_hw(x, weight, r1=1, r2=33, iters=10):
    """Estimate true HW kernel time by wall-clock differencing of two
    repeat-loop variants: T = (wall(r2) - wall(r1)) / (r2 - r1). Cancels the
    ~100 ms axon RPC dispatch floor. Returns (out, hw_seconds_estimate)."""
    out, t1 = bench(x, weight, iters=iters, repeat=r1)
    _, t2 = bench(x, weight, iters=iters, repeat=r2)
    hw = (min(t2) - min(t1)) / (r2 - r1)
    return out, hw


# revision 14
# speedup vs baseline: 116.8860x; 2.9744x over previous
"""Trainium2 Bass kernel: single-channel 11x11 same-padding 2D cross-correlation.

Problem: x [64, 1024, 1024] f32, weight [11, 11] f32 ->
         out[b,h,w] = sum_{i,j} x_pad[b, h+i-5, w+j-5] * weight[i,j]

Strategy
--------
Pure data parallel over batch: 8 images per NeuronCore across 8 cores.

Per core, the conv is computed on the TensorEngine as banded-Toeplitz
matmuls. For an output row-tile of MTILE=118 rows, the contraction dim
(SBUF partition axis) holds 128 input rows (118 + 10 halo). For each of
the 11 kernel columns j we issue one matmul:

    psum[m, n] += sum_p T_j[p, m] * xtile[p, n + j]

where T_j[p, m] = weight[p - m, j] for 0 <= p - m <= 10 (banded, built
host-side), and the rhs access pattern is just the x tile shifted by j
along the free (W) axis. 11 matmuls accumulate the full 11x11 stencil
into one PSUM tile. x is host-padded (zeros) so every DMA and matmul is
uniform: H padded by 5 on top (and to the tile grid at the bottom), W
padded by 5 on both sides.

dtype: float32r (reduced-precision fp32 matmul, 1 col/cycle at N>=256)
or bf16 (host-cast, halves input DMA bytes). PSUM accumulation is fp32
either way.
"""

import math

import numpy as np

KK = 11      # kernel size
PAD = 5      # same padding
MTILE = 118  # output rows per tile; contraction = MTILE + 2*PAD = 128
KDIM = 128   # contraction partitions
NCORES = 8

# "fp32r" | "bf16" | "fp32"
DTYPE = "bf16"

_CACHE = {}


def build_tmats(weight, dtype_np):
    """[128, 11*118] stationary banded matrices; T_j columns m, band = kernel col j."""
    T = np.zeros((KDIM, KK * MTILE), dtype=np.float32)
    for j in range(KK):
        for d in range(KK):
            # T[m + d, j*MTILE + m] = weight[d, j]
            idx_m = np.arange(0, MTILE)
            idx_p = idx_m + d
            ok = idx_p < KDIM
            T[idx_p[ok], j * MTILE + idx_m[ok]] = weight[d, j]
    return np.ascontiguousarray(T.astype(dtype_np))


def _dt():
    import concourse.mybir as mybir
    import ml_dtypes

    if DTYPE == "fp32r":
        return mybir.dt.float32r, np.float32
    if DTYPE == "bf16":
        return mybir.dt.bfloat16, ml_dtypes.bfloat16
    return mybir.dt.float32, np.float32


def build_nc(b, h, w, repeat=1):
    """Bass program for one core: b images of [h, w].

    repeat > 1 wraps the whole body in a hardware For-loop that redoes the
    identical work; used only for wall-clock-delta HW timing (the axon RPC
    dispatch floor is ~100 ms, far above the kernel's real runtime).
    """
    import contextlib

    import concourse.mybir as mybir
    from concourse import bacc
    from concourse.tile import TileContext

    dt_mm, _ = _dt()
    ntiles = math.ceil(h / MTILE)
    hp = MTILE * (ntiles - 1) + KDIM   # host-padded H
    wp = w + 2 * PAD                   # host-padded W
    nhalf = w // 512
    assert w % 512 == 0

    nc = bacc.Bacc("TRN2", target_bir_lowering=False)
    x = nc.dram_tensor("x", (b, hp, wp), dt_mm, kind="ExternalInput")
    tm = nc.dram_tensor("tmats", (KDIM, KK * MTILE), dt_mm, kind="ExternalInput")
    out = nc.dram_tensor("out", (b, h, w), mybir.dt.float32, kind="ExternalOutput")

    with TileContext(nc) as tc:
        with (
            tc.tile_pool(name="wpool", bufs=1) as wpool,
            tc.tile_pool(name="xpool", bufs=3) as xpool,
            tc.tile_pool(name="opool", bufs=3) as opool,
            tc.tile_pool(name="psum", bufs=4, space="PSUM") as ppool,
            tc.tile_pool(name="scratch", bufs=1, space="PSUM") as spool,
        ):
            tsb = wpool.tile([KDIM, KK * MTILE], dt_mm)
            nc.sync.dma_start(tsb[:, :], tm[:, :])
            scr = spool.tile([1, 8], mybir.dt.float32)
            loop = tc.For_i(0, repeat, 1) if repeat > 1 else contextlib.nullcontext()
            with loop:
                for img in range(b):
                    for t in range(ntiles):
                        a = t * MTILE
                        xt = xpool.tile([KDIM, wp], dt_mm)
                        nc.sync.dma_start(xt[:, :], x[img, a:a + KDIM, :])
                        # Pre-touch: a 1x1 dummy matmul absorbs the
                        # DMA-completion wait on the PE queue, so real matmuls
                        # (whose fused weight-load struct has a single
                        # sync-wait slot) never carry more than one wait each.
                        nc.tensor.matmul(
                            scr[0:1, 0:2], xt[0:1, 0:1], xt[0:1, 0:2],
                            start=True, stop=True, skip_group_check=True,
                        )
                        ot = opool.tile([MTILE, w], mybir.dt.float32)
                        for half in range(nhalf):
                            ps = ppool.tile([MTILE, 512], mybir.dt.float32)
                            base = half * 512
                            for j in range(KK):
                                nc.tensor.matmul(
                                    ps[:, :],
                                    tsb[:, j * MTILE:(j + 1) * MTILE],
                                    xt[:, base + j: base + j + 512],
                                    start=(j == 0),
                                    stop=(j == KK - 1),
                                )
                            nc.vector.tensor_copy(ot[:, base:base + 512], ps[:, :])
                        rows = min(MTILE, h - a)
                        nc.sync.dma_start(out[img, a:a + rows, :], ot[:rows, :])
    nc.compile()
    return nc


def _pad_input(x, h, w, dtype_np):
    """[B, hp, wp] zero-padded copy of x."""
    B = x.shape[0]
    ntiles = math.ceil(h / MTILE)
    hp = MTILE * (ntiles - 1) + KDIM
    wp = w + 2 * PAD
    xp = np.zeros((B, hp, wp), dtype=dtype_np)
    xp[:, PAD:PAD + h, PAD:PAD + w] = x
    return xp


def kernel(x, weight):
    from concourse.bass_utils import run_bass_kernel_spmd

    x = np.asarray(x)
    weight = np.asarray(weight)
    B, h, w = x.shape
    assert B % NCORES == 0
    bpc = B // NCORES
    _, dtype_np = _dt()

    key = (bpc, h, w, DTYPE, 1)
    if key not in _CACHE:
        _CACHE[key] = build_nc(bpc, h, w)
    nc = _CACHE[key]

    xp = _pad_input(x, h, w, dtype_np)
    tm = build_tmats(weight.astype(np.float32), dtype_np)
    in_maps = [
        {"x": xp[c * bpc:(c + 1) * bpc], "tmats": tm} for c in range(NCORES)
    ]
    res = run_bass_kernel_spmd(nc, in_maps, core_ids=list(range(NCORES)))
    global _LAST_RESULTS
    _LAST_RESULTS = res
    return np.concatenate([r["out"] for r in res.results], axis=0).astype(np.float32)


def bench(x, weight, iters=20, repeat=1):
    """Time device execution with device-resident inputs (no donation, no
    per-iter host transfers). Returns (out, per-iter seconds list)."""
    import time

    import jax
    from jax.experimental.shard_map import shard_map
    from jax.sharding import Mesh, PartitionSpec

    import concourse.mybir as mybir
    from concourse import bass2jax

    x = np.asarray(x)
    weight = np.asarray(weight)
    B, h, w = x.shape
    bpc = B // NCORES
    _, dtype_np = _dt()
    key = (bpc, h, w, DTYPE, repeat)
    if key not in _CACHE:
        _CACHE[key] = build_nc(bpc, h, w, repeat=repeat)
    nc = _CACHE[key]

    bass2jax.install_neuronx_cc_hook()
    partition_name = nc.partition_id_tensor.name if nc.partition_id_tensor else None
    in_names, out_names, out_avals = [], [], []
    for alloc in nc.m.functions[0].allocations:
        if not isinstance(alloc, mybir.MemoryLocationSet):
            continue
        name = alloc.memorylocations[0].name
        if alloc.kind == "ExternalInput":
            if name != partition_name:
                in_names.append(name)
        elif alloc.kind == "ExternalOutput":
            out_names.append(name)
            out_avals.append(
                jax.core.ShapedArray(
                    tuple(alloc.tensor_shape), mybir.dt.np(alloc.dtype)
                )
            )
    n_params = len(in_names)
    all_in_names = in_names + out_names
    if partition_name is not None:
        all_in_names = all_in_names + [partition_name]

    def _body(*args):
        operands = list(args)
        if partition_name is not None:
            operands.append(bass2jax.partition_id_tensor())
        return tuple(
            bass2jax._bass_exec_p.bind(
                *operands,
                out_avals=tuple(out_avals),
                in_names=tuple(all_in_names),
                out_names=tuple(out_names),
                lowering_input_output_aliases=(),
                sim_require_finite=True,
                sim_require_nnan=True,
                nc=nc,
            )
        )

    devices = jax.devices()[:NCORES]
    mesh = Mesh(np.asarray(devices), ("core",))
    n_outs = len(out_names)
    fn = jax.jit(
        shard_map(
            _body,
            mesh=mesh,
            in_specs=(PartitionSpec("core"),) * (n_params + n_outs),
            out_specs=(PartitionSpec("core"),) * n_outs,
            check_rep=False,
        ),
        keep_unused=True,
    )

    xp = _pad_input(x, h, w, dtype_np)
    tm = build_tmats(weight.astype(np.float32), dtype_np)
    per_core = {"x": xp, "tmats": np.concatenate([tm[None]] * NCORES, 0).reshape(NCORES * tm.shape[0], tm.shape[1])}
    concat_in = [per_core[name] for name in in_names]
    concat_zeros = [
        np.zeros((NCORES * a.shape[0], *a.shape[1:]), a.dtype) for a in out_avals
    ]
    from jax.sharding import NamedSharding
    shard = NamedSharding(mesh, PartitionSpec("core"))
    dev_in = [jax.device_put(a, shard) for a in concat_in]
    dev_zero = [jax.device_put(a, shard) for a in concat_zeros]

    out = fn(*dev_in, *dev_zero)  # compile + warmup
    jax.block_until_ready(out)
    times = []
    for _ in range(iters):
        t0 = time.perf_counter()
        out = fn(*dev_in, *dev_zero)
        jax.block_until_ready(out)
        times.append(time.perf_counter() - t0)
    full = np.asarray(out[0]).reshape(NCORES, bpc, h, w).reshape(B, h, w)
    return full.astype(np.float32), times


def bench_hw(x, weight, r1=1, r2=33, iters=10):
    """Estimate true HW kernel time by wall-clock differencing of two
    repeat-loop variants: T = (wall(r2) - wall(r1)) / (r2 - r1). Cancels the
    ~100 ms axon RPC dispatch floor. Returns (out, hw_seconds_estimate)."""
    out, t1 = bench(x, weight, iters=iters, repeat=r1)
    _, t2 = bench(x, weight, iters=iters, repeat=r2)
    hw = (min(t2) - min(t1)) / (r2 - r1)
    return out, hw
